# revision 12
# baseline (speedup 1.0000x reference)
"""Bass/Tile TRN2 kernel for BasicAttention.

att = softmax(tanh(hidden @ W_h.T + p_att_feats) @ W_alpha + mask) @ att_feats

Shapes: B=64, N=2048, H=1024, A=512. Data-parallel over batch across 8
NeuronCores (8 batches per core); weights replicated; no collectives.

Layout: region index n maps to (partition p, column c) as n = p*16 + c so
every p_att/att_feats DMA is a long contiguous 16KB-per-partition read and
the mask tile is a natural row-major reshape.

The kernel is DMA-bound: all 16 SDMA engines must stay fed (aggregate
~430 GB/s). The stream DMAs are split across BOTH HWDGE rings with equal
bytes per batch so the rings drain together and each ring's per-DMA
turnaround hides behind the other ring's packets:
  ring nc.sync   (SP):  p_att chunk 0 + att_feats cols 0-7   (6 MiB/batch)
  ring nc.scalar (ACT): p_att chunk 1 + att_feats cols 8-15  (6 MiB/batch)
  nc.gpsimd (SWDGE): output row stores (can never block a stream ring)
Consts (W_hT, hidden_T, W_alpha, masks, E) load once up front on the ACT
ring; the per-batch w_h broadcast is built on-device with PE indicator
matmuls (no DRAM round-trip, no HBM re-reads).

Pipeline is emitted so no engine's in-order stream ever parks on a
not-yet-ready input while later DMA dispatches sit behind it:
  iter b: patt_phase(b)   DMAs, adds, tanhs, dot-accum, exp  (stage-major)
          af_main(b-1)    sum+recip, af DMAs + PE matmuls in arrival order
          af_finish(b-2)  scale + store (PE work long since done)
"""

import numpy as np

B, N, H, A = 64, 2048, 1024, 512
NCORES = 8
BLOC = B // NCORES  # batches per core

P = 128
NT = N // P            # 16 n-columns per partition
PATT_SUP = 8           # columns per p_att chunk (2MiB, 2 DMAs per batch)
AF_SUP = 4             # columns per att_feats chunk (2MiB, 4 DMAs per batch)

_NC_CACHE = {}


def _free_bcast(bass_mod, ap, repeat):
    """[P, F] AP -> [P, repeat, F] AP with 0-stride middle dim."""
    return bass_mod.AP(
        tensor=ap.tensor,
        offset=ap.offset,
        ap=[ap.ap[0], [0, repeat], *ap.ap[1:]],
    )


def _build_nc():
    import concourse.bass as bass
    import concourse.mybir as mybir
    import concourse.tile as tile
    from concourse import bacc

    dt = mybir.dt
    f32, f32r, bf16 = dt.float32, dt.float32r, dt.bfloat16
    AF = mybir.ActivationFunctionType
    OP = mybir.AluOpType

    nc = bacc.Bacc("TRN2", target_bir_lowering=False, debug=False,
                   num_devices=NCORES)

    NBC = BLOC + 1  # 8 w_h rows + 1 W_alpha row to broadcast

    hsT = nc.dram_tensor("hidden_T", [H, BLOC], f32, kind="ExternalInput").ap()
    af = nc.dram_tensor("att_feats", [BLOC, N, H], f32r, kind="ExternalInput").ap()
    pa = nc.dram_tensor("p_att_feats", [BLOC, N, A], f32, kind="ExternalInput").ap()
    am = nc.dram_tensor("att_masks", [BLOC, N], f32, kind="ExternalInput").ap()
    whT = nc.dram_tensor("W_hT", [H, A], f32, kind="ExternalInput").ap()
    war = nc.dram_tensor("W_alpha_r", [1, A], f32, kind="ExternalInput").ap()
    ebc = nc.dram_tensor("E_bc", [NBC, NBC * P], f32, kind="ExternalInput").ap()
    out = nc.dram_tensor("att_out", [BLOC, H], f32, kind="ExternalOutput").ap()

    HC = H // P  # 8 contraction chunks for the w_h matmul

    with tile.TileContext(nc) as tc:
        with (
            tc.tile_pool(name="consts", bufs=1) as consts,
            tc.tile_pool(name="patt", bufs=4) as patt_pool,
            tc.tile_pool(name="alpha", bufs=3) as alpha_pool,
            tc.tile_pool(name="afp", bufs=5) as af_pool,
            tc.tile_pool(name="small", bufs=4) as small,
            tc.tile_pool(name="rowp", bufs=2) as rowp,
            tc.tile_pool(name="psmisc", bufs=1, space="PSUM") as psmisc,
            tc.tile_pool(name="psbc", bufs=3, space="PSUM") as psbc,
            tc.tile_pool(name="psatt", bufs=4, space="PSUM") as psatt,
        ):
            # ---------------- setup ----------------
            ones_col = consts.tile([P, 1], f32)
            nc.vector.memset(ones_col, 1.0)

            # indicator matrix for PE row-broadcast: E[:, b*128:(b+1)*128]
            # has row b all-ones, so E_b^T @ whall (K=9) replicates whall's
            # row b onto all 128 output partitions
            eall = consts.tile([NBC, NBC * P], f32)
            nc.sync.dma_start(out=eall, in_=ebc)
            whall = consts.tile([NBC, A], f32, name="whall", tag="whall")

            # consts split across BOTH rings with equal bytes so the two
            # stream rings start phase-aligned; whT borrows alpha-pool slots
            # (same 8KB/partition) so the af stream never waits on them
            whT_r = whT.rearrange("(c p) a -> p c a", p=P)
            whT_sb = []
            for i, eng in enumerate([nc.sync, nc.scalar]):
                t = alpha_pool.tile([P, HC // 2, A], f32, name=f"whT{i}",
                                    tag="alpha")
                eng.dma_start(out=t, in_=whT_r[:, i * 4:(i + 1) * 4, :])
                whT_sb.append(t)
            hidT_sb = consts.tile([P, HC, BLOC], f32)
            nc.sync.dma_start(out=hidT_sb,
                              in_=hsT.rearrange("(c p) b -> p c b", p=P))
            nc.sync.dma_start(out=whall[BLOC:BLOC + 1, :], in_=war)
            masks_sb = consts.tile([P, BLOC, NT], f32)
            nc.scalar.dma_start(out=masks_sb,
                                in_=am.rearrange("b (p c) -> p b c", c=NT))

            # w_h = hidden @ W_h.T : [8, 512]
            wh_ps = psmisc.tile([BLOC, A], f32, tag="mm")
            for hc in range(HC):
                nc.tensor.matmul(wh_ps, lhsT=hidT_sb[:, hc, :],
                                 rhs=whT_sb[hc // 4][:, hc % 4, :],
                                 start=(hc == 0), stop=(hc == HC - 1))
            nc.vector.tensor_copy(whall[0:BLOC, :], wh_ps)

            # PE broadcast: w_h row b -> whb[b] [128, 512] f32 (and W_alpha
            # -> [128, 512] bf16), no HBM traffic
            whb = []
            for b in range(BLOC):
                ps = psbc.tile([P, A], f32, tag="bc", name=f"whbps{b}")
                nc.tensor.matmul(ps, lhsT=eall[:, b * P:(b + 1) * P],
                                 rhs=whall, start=True, stop=True)
                t = consts.tile([P, A], f32, name=f"whb{b}", tag=f"whb{b}")
                nc.vector.tensor_copy(t, ps)
                whb.append(t)
            wa_ps = psbc.tile([P, A], f32, tag="bc", name="waps")
            nc.tensor.matmul(wa_ps, lhsT=eall[:, BLOC * P:NBC * P],
                             rhs=whall, start=True, stop=True)
            wa_bf = consts.tile([P, A], bf16)
            nc.vector.tensor_copy(wa_bf, wa_ps)

            # ---------------- main loop (software-pipelined) ----------------
            # n = p*NT + c everywhere below.
            pa_r = [pa[b, :, :].rearrange("(p c) a -> p c a", c=NT)
                    for b in range(BLOC)]
            af_r = [af[b, :, :].rearrange("(p c) h -> p c h", c=NT)
                    for b in range(BLOC)]

            NST = NT // PATT_SUP       # 2 p_att chunks per batch
            patt_eng = [nc.sync, nc.scalar]

            def patt_phase(b):
                # stage-major emission: DMAs, adds, tanhs, dot-accumulates —
                # DVE and ACT pipeline across chunks instead of serializing
                pts, abs_ = [], []
                for st in range(NST):
                    pt = patt_pool.tile([P, PATT_SUP, A], f32, tag="patt",
                                        name=f"patt{b}_{st}")
                    patt_eng[st].dma_start(
                        out=pt,
                        in_=pa_r[b][:, st * PATT_SUP:(st + 1) * PATT_SUP, :],
                    )
                    pts.append(pt)
                whb_b = _free_bcast(bass, whb[b][:, :], PATT_SUP)
                for st in range(NST):
                    nc.vector.tensor_tensor(out=pts[st], in0=pts[st],
                                            in1=whb_b, op=OP.add)
                for st in range(NST):
                    ab = alpha_pool.tile([P, PATT_SUP, A], bf16, tag="alpha",
                                         name=f"alpha{b}_{st}")
                    nc.scalar.activation(ab, pts[st], AF.Tanh)
                    abs_.append(ab)
                scores = small.tile([P, NT], f32, tag="scores",
                                    name=f"scores{b}")
                for st in range(NST):
                    for c in range(PATT_SUP):
                        col = st * PATT_SUP + c
                        # out = (ab * 1) * wa ; accum_out = row-sum -> scores
                        nc.vector.scalar_tensor_tensor(
                            out=abs_[st][:, c, :], in0=abs_[st][:, c, :],
                            scalar=1.0, in1=wa_bf, op0=OP.mult, op1=OP.mult,
                            accum_out=scores[:, col:col + 1],
                        )

                nc.vector.tensor_tensor(out=scores, in0=scores,
                                        in1=masks_sb[:, b, :], op=OP.add)

                expt = small.tile([P, NT], f32r, tag="expt", name=f"expt{b}")
                rowsum = small.tile([P, 1], f32, tag="rowsum", name=f"rowsum{b}")
                nc.scalar.activation(expt, scores, AF.Exp, accum_out=rowsum)
                return expt, rowsum

            def af_main(b, expt, rowsum):
                # total-sum + reciprocal live here so the PE stream reaches
                # them only when exp(b) is already done
                sum_ps = psmisc.tile([1, 1], f32, tag="mm", name=f"sum_ps{b}")
                nc.tensor.matmul(sum_ps, lhsT=rowsum, rhs=ones_col,
                                 start=True, stop=True)
                inv = small.tile([1, 1], f32, tag="inv", name=f"inv{b}")
                nc.vector.reciprocal(inv, sum_ps)

                att_lo = psatt.tile([1, A], f32, tag="att", name=f"attlo{b}")
                att_hi = psatt.tile([1, A], f32, tag="att", name=f"atthi{b}")
                # chunks listed in ring-arrival order (sync/scalar alternate);
                # the last batch tapers so the post-last-byte tail is tiny
                if b < BLOC - 1:
                    chunks = [(nc.sync, 0, 4), (nc.scalar, 8, 4),
                              (nc.sync, 4, 4), (nc.scalar, 12, 4)]
                else:
                    chunks = [(nc.sync, 0, 4), (nc.scalar, 8, 4),
                              (nc.sync, 4, 4), (nc.scalar, 12, 2),
                              (nc.scalar, 14, 1), (nc.scalar, 15, 1)]
                nch = len(chunks)
                for ci, (eng, c0, sz) in enumerate(chunks):
                    aft = af_pool.tile([P, sz, H], f32r, tag="af",
                                       name=f"af{b}_{ci}")
                    eng.dma_start(out=aft, in_=af_r[b][:, c0:c0 + sz, :])
                    for c in range(sz):
                        first = ci == 0 and c == 0
                        last = ci == nch - 1 and c == sz - 1
                        lhs = expt[:, c0 + c:c0 + c + 1]
                        nc.tensor.matmul(att_lo, lhsT=lhs,
                                         rhs=aft[:, c, 0:A],
                                         start=first, stop=last)
                        nc.tensor.matmul(att_hi, lhsT=lhs,
                                         rhs=aft[:, c, A:H],
                                         start=first, stop=last)
                return att_lo, att_hi, inv

            def af_finish(b, att_lo, att_hi, inv):
                att_row = rowp.tile([1, H], f32, tag="attrow",
                                    name=f"attrow{b}")
                nc.vector.tensor_scalar_mul(att_row[:, 0:A], att_lo, inv)
                nc.vector.tensor_scalar_mul(att_row[:, A:H], att_hi, inv)
                nc.gpsimd.dma_start(out=out[b:b + 1, :], in_=att_row)

            st, fin = {}, {}
            for b in range(BLOC):
                st[b] = patt_phase(b)
                if b >= 1:
                    fin[b - 1] = af_main(b - 1, *st.pop(b - 1))
                if b >= 2:
                    af_finish(b - 2, *fin.pop(b - 2))
            fin[BLOC - 1] = af_main(BLOC - 1, *st.pop(BLOC - 1))
            af_finish(BLOC - 2, *fin.pop(BLOC - 2))
            af_finish(BLOC - 1, *fin.pop(BLOC - 1))

    nc.compile()
    return nc


def _get_nc():
    if "nc" not in _NC_CACHE:
        _NC_CACHE["nc"] = _build_nc()
    return _NC_CACHE["nc"]


def kernel(hidden_states, att_feats, p_att_feats, att_masks, W_h, W_alpha):
    from concourse.bass_utils import run_bass_kernel_spmd

    nc = _get_nc()
    hidden_states = np.ascontiguousarray(hidden_states, dtype=np.float32)
    att_feats = np.ascontiguousarray(att_feats, dtype=np.float32)
    p_att_feats = np.ascontiguousarray(p_att_feats, dtype=np.float32)
    att_masks = np.ascontiguousarray(att_masks, dtype=np.float32)
    W_h = np.ascontiguousarray(W_h, dtype=np.float32)
    W_alpha = np.ascontiguousarray(
        np.asarray(W_alpha, dtype=np.float32).reshape(1, A))

    whT = np.ascontiguousarray(W_h.T)                       # [H, A]
    nbc = BLOC + 1
    ebc = np.zeros((nbc, nbc * P), dtype=np.float32)
    for b in range(nbc):
        ebc[b, b * P:(b + 1) * P] = 1.0

    in_maps = []
    for i in range(NCORES):
        s = slice(i * BLOC, (i + 1) * BLOC)
        in_maps.append({
            "hidden_T": np.ascontiguousarray(hidden_states[s].T),
            "att_feats": att_feats[s],
            "p_att_feats": p_att_feats[s],
            "att_masks": att_masks[s],
            "W_hT": whT,
            "W_alpha_r": W_alpha,
            "E_bc": ebc,
        })

    global _LAST_IN_MAPS
    _LAST_IN_MAPS = in_maps
    res = run_bass_kernel_spmd(nc, in_maps, core_ids=list(range(NCORES)))
    return np.concatenate(
        [res.results[i]["att_out"] for i in range(NCORES)], axis=0
    ).astype(np.float32)


_LAST_IN_MAPS = None


# revision 15
# speedup vs baseline: 1.2705x; 1.2705x over previous
"""Bass/Tile TRN2 kernel for BasicAttention.

att = softmax(tanh(hidden @ W_h.T + p_att_feats) @ W_alpha + mask) @ att_feats

Shapes: B=64, N=2048, H=1024, A=512. Data-parallel over batch across 8
NeuronCores (8 batches per core); weights replicated; no collectives.

Layout: region index n maps to (partition p, column c) as n = p*16 + c so
every p_att/att_feats DMA is a long contiguous 16KB-per-partition read and
the mask tile is a natural row-major reshape.

The kernel is DMA-bound: all 16 SDMA engines must stay fed (aggregate
~430 GB/s). The stream DMAs are split across BOTH HWDGE rings with equal
bytes per batch so the rings drain together and each ring's per-DMA
turnaround hides behind the other ring's packets:
  ring nc.sync   (SP):  p_att chunk 0 + att_feats cols 0-7   (6 MiB/batch)
  ring nc.scalar (ACT): p_att chunk 1 + att_feats cols 8-15  (6 MiB/batch)
  nc.gpsimd (SWDGE): output row stores (can never block a stream ring)
Consts (W_hT, hidden_T, W_alpha, masks, E) load once up front on the ACT
ring; the per-batch w_h broadcast is built on-device with PE indicator
matmuls (no DRAM round-trip, no HBM re-reads).

Pipeline is emitted so no engine's in-order stream ever parks on a
not-yet-ready input while later DMA dispatches sit behind it:
  iter b: patt_phase(b)   DMAs, adds, tanhs, dot-accum, exp  (stage-major)
          af_main(b-1)    sum+recip, af DMAs + PE matmuls in arrival order
          af_finish(b-2)  scale + store (PE work long since done)
"""

import numpy as np

B, N, H, A = 64, 2048, 1024, 512
NCORES = 8
BLOC = B // NCORES  # batches per core

P = 128
NT = N // P            # 16 n-columns per partition
PATT_SUP = 8           # columns per p_att chunk (2MiB, 2 DMAs per batch)
AF_SUP = 4             # columns per att_feats chunk (2MiB, 4 DMAs per batch)

_NC_CACHE = {}


def _free_bcast(bass_mod, ap, repeat):
    """[P, F] AP -> [P, repeat, F] AP with 0-stride middle dim."""
    return bass_mod.AP(
        tensor=ap.tensor,
        offset=ap.offset,
        ap=[ap.ap[0], [0, repeat], *ap.ap[1:]],
    )


def _build_nc():
    import concourse.bass as bass
    import concourse.mybir as mybir
    import concourse.tile as tile
    from concourse import bacc

    dt = mybir.dt
    f32, f32r, bf16 = dt.float32, dt.float32r, dt.bfloat16
    AF = mybir.ActivationFunctionType
    OP = mybir.AluOpType

    nc = bacc.Bacc("TRN2", target_bir_lowering=False, debug=False,
                   num_devices=NCORES)

    NBC = BLOC + 1  # 8 w_h rows + 1 W_alpha row to broadcast

    hsT = nc.dram_tensor("hidden_T", [H, BLOC], f32, kind="ExternalInput").ap()
    af = nc.dram_tensor("att_feats", [BLOC, N, H], f32r, kind="ExternalInput").ap()
    pa = nc.dram_tensor("p_att_feats", [BLOC, N, A], f32, kind="ExternalInput").ap()
    am = nc.dram_tensor("att_masks", [BLOC, N], f32, kind="ExternalInput").ap()
    whT = nc.dram_tensor("W_hT", [H, A], f32, kind="ExternalInput").ap()
    war = nc.dram_tensor("W_alpha_r", [1, A], f32, kind="ExternalInput").ap()
    ebc = nc.dram_tensor("E_bc", [NBC, NBC * P], f32, kind="ExternalInput").ap()
    out = nc.dram_tensor("att_out", [BLOC, H], f32, kind="ExternalOutput").ap()

    HC = H // P  # 8 contraction chunks for the w_h matmul

    with tile.TileContext(nc) as tc:
        with (
            tc.tile_pool(name="consts", bufs=1) as consts,
            tc.tile_pool(name="patt", bufs=3) as patt_pool,
            tc.tile_pool(name="alpha", bufs=3) as alpha_pool,
            tc.tile_pool(name="afp", bufs=5) as af_pool,
            tc.tile_pool(name="small", bufs=4) as small,
            tc.tile_pool(name="psmisc", bufs=1, space="PSUM") as psmisc,
            tc.tile_pool(name="psbc", bufs=3, space="PSUM") as psbc,
            tc.tile_pool(name="psatt", bufs=4, space="PSUM") as psatt,
        ):
            # ---------------- setup ----------------
            ones_col = consts.tile([P, 1], f32)
            nc.vector.memset(ones_col, 1.0)

            # indicator matrix for PE row-broadcast: E[:, b*128:(b+1)*128]
            # has row b all-ones, so E_b^T @ whall (K=9) replicates whall's
            # row b onto all 128 output partitions
            eall = consts.tile([NBC, NBC * P], f32)
            nc.sync.dma_start(out=eall, in_=ebc)
            whall = consts.tile([NBC, A], f32, name="whall", tag="whall")

            # consts split across BOTH rings with equal bytes so the two
            # stream rings start phase-aligned; whT borrows af-pool slots
            whT_r = whT.rearrange("(c p) a -> p c a", p=P)
            whT_sb = []
            for i, eng in enumerate([nc.sync, nc.scalar]):
                t = af_pool.tile([P, HC // 2, A], f32, name=f"whT{i}",
                                 tag="af")
                eng.dma_start(out=t, in_=whT_r[:, i * 4:(i + 1) * 4, :])
                whT_sb.append(t)
            hidT_sb = consts.tile([P, HC, BLOC], f32)
            nc.sync.dma_start(out=hidT_sb,
                              in_=hsT.rearrange("(c p) b -> p c b", p=P))
            nc.sync.dma_start(out=whall[BLOC:BLOC + 1, :], in_=war)
            masks_sb = consts.tile([P, BLOC, NT], f32)
            nc.scalar.dma_start(out=masks_sb,
                                in_=am.rearrange("b (p c) -> p b c", c=NT))

            # w_h = hidden @ W_h.T : [8, 512]
            wh_ps = psmisc.tile([BLOC, A], f32, tag="mm")
            for hc in range(HC):
                nc.tensor.matmul(wh_ps, lhsT=hidT_sb[:, hc, :],
                                 rhs=whT_sb[hc // 4][:, hc % 4, :],
                                 start=(hc == 0), stop=(hc == HC - 1))
            nc.vector.tensor_copy(whall[0:BLOC, :], wh_ps)

            # PE broadcast: w_h row b -> whb[b] [128, 512] f32 (and W_alpha
            # -> [128, 512] bf16), no HBM traffic
            whb = []
            for b in range(BLOC):
                ps = psbc.tile([P, A], f32, tag="bc", name=f"whbps{b}")
                nc.tensor.matmul(ps, lhsT=eall[:, b * P:(b + 1) * P],
                                 rhs=whall, start=True, stop=True)
                t = consts.tile([P, A], f32, name=f"whb{b}", tag=f"whb{b}")
                nc.vector.tensor_copy(t, ps)
                whb.append(t)
            wa_ps = psbc.tile([P, A], f32, tag="bc", name="waps")
            nc.tensor.matmul(wa_ps, lhsT=eall[:, BLOC * P:NBC * P],
                             rhs=whall, start=True, stop=True)
            wa_bf = consts.tile([P, A], bf16)
            nc.vector.tensor_copy(wa_bf, wa_ps)

            # ---------------- main loop (software-pipelined) ----------------
            # n = p*NT + c everywhere below.
            pa_r = [pa[b, :, :].rearrange("(p c) a -> p c a", c=NT)
                    for b in range(BLOC)]
            af_r = [af[b, :, :].rearrange("(p c) h -> p c h", c=NT)
                    for b in range(BLOC)]

            NST = NT // PATT_SUP       # 2 p_att chunks per batch
            patt_eng = [nc.sync, nc.scalar]

            def patt_phase(b):
                # stage-major emission: DMAs, adds, tanhs, dot-accumulates —
                # DVE and ACT pipeline across chunks instead of serializing
                pts, abs_ = [], []
                for st in range(NST):
                    pt = patt_pool.tile([P, PATT_SUP, A], f32, tag="patt",
                                        name=f"patt{b}_{st}")
                    patt_eng[st].dma_start(
                        out=pt,
                        in_=pa_r[b][:, st * PATT_SUP:(st + 1) * PATT_SUP, :],
                    )
                    pts.append(pt)
                whb_b = _free_bcast(bass, whb[b][:, :], PATT_SUP)
                for st in range(NST):
                    nc.vector.tensor_tensor(out=pts[st], in0=pts[st],
                                            in1=whb_b, op=OP.add)
                for st in range(NST):
                    ab = alpha_pool.tile([P, PATT_SUP, A], bf16, tag="alpha",
                                         name=f"alpha{b}_{st}")
                    nc.scalar.activation(ab, pts[st], AF.Tanh)
                    abs_.append(ab)
                scores = small.tile([P, NT], f32, tag="scores",
                                    name=f"scores{b}")
                for st in range(NST):
                    for c in range(PATT_SUP):
                        col = st * PATT_SUP + c
                        # out = (ab * 1) * wa ; accum_out = row-sum -> scores
                        nc.vector.scalar_tensor_tensor(
                            out=abs_[st][:, c, :], in0=abs_[st][:, c, :],
                            scalar=1.0, in1=wa_bf, op0=OP.mult, op1=OP.mult,
                            accum_out=scores[:, col:col + 1],
                        )

                nc.vector.tensor_tensor(out=scores, in0=scores,
                                        in1=masks_sb[:, b, :], op=OP.add)

                expt = small.tile([P, NT], f32r, tag="expt", name=f"expt{b}")
                rowsum = small.tile([P, 1], f32, tag="rowsum", name=f"rowsum{b}")
                nc.scalar.activation(expt, scores, AF.Exp, accum_out=rowsum)
                return expt, rowsum

            def af_main(b, expt, rowsum):
                # total-sum + reciprocal live here so the PE stream reaches
                # them only when exp(b) is already done
                sum_ps = psmisc.tile([1, 1], f32, tag="mm", name=f"sum_ps{b}")
                nc.tensor.matmul(sum_ps, lhsT=rowsum, rhs=ones_col,
                                 start=True, stop=True)
                inv = small.tile([1, 1], f32, tag="inv", name=f"inv{b}")
                nc.vector.reciprocal(inv, sum_ps)

                att_lo = psatt.tile([1, A], f32, tag="att", name=f"attlo{b}")
                att_hi = psatt.tile([1, A], f32, tag="att", name=f"atthi{b}")
                # chunks listed in ring-arrival order (sync/scalar alternate);
                # the last batch tapers so the post-last-byte tail is tiny
                if b < BLOC - 1:
                    chunks = [(nc.sync, 0, 4), (nc.scalar, 8, 4),
                              (nc.sync, 4, 4), (nc.scalar, 12, 4)]
                else:
                    chunks = [(nc.sync, 0, 4), (nc.scalar, 8, 4),
                              (nc.sync, 4, 4), (nc.scalar, 12, 2),
                              (nc.scalar, 14, 1), (nc.scalar, 15, 1)]
                nch = len(chunks)
                for ci, (eng, c0, sz) in enumerate(chunks):
                    aft = af_pool.tile([P, sz, H], f32r, tag="af",
                                       name=f"af{b}_{ci}")
                    eng.dma_start(out=aft, in_=af_r[b][:, c0:c0 + sz, :])
                    for c in range(sz):
                        first = ci == 0 and c == 0
                        last = ci == nch - 1 and c == sz - 1
                        lhs = expt[:, c0 + c:c0 + c + 1]
                        nc.tensor.matmul(att_lo, lhsT=lhs,
                                         rhs=aft[:, c, 0:A],
                                         start=first, stop=last)
                        nc.tensor.matmul(att_hi, lhsT=lhs,
                                         rhs=aft[:, c, A:H],
                                         start=first, stop=last)
                return att_lo, att_hi, inv

            def af_finish(b, att_lo, att_hi, inv):
                att_row = small.tile([1, H], f32, tag="attrow",
                                     name=f"attrow{b}")
                nc.vector.tensor_scalar_mul(att_row[:, 0:A], att_lo, inv)
                nc.vector.tensor_scalar_mul(att_row[:, A:H], att_hi, inv)
                nc.gpsimd.dma_start(out=out[b:b + 1, :], in_=att_row)

            st, fin = {}, {}
            for b in range(BLOC):
                st[b] = patt_phase(b)
                if b >= 1:
                    fin[b - 1] = af_main(b - 1, *st.pop(b - 1))
                if b >= 2:
                    af_finish(b - 2, *fin.pop(b - 2))
            fin[BLOC - 1] = af_main(BLOC - 1, *st.pop(BLOC - 1))
            af_finish(BLOC - 2, *fin.pop(BLOC - 2))
            af_finish(BLOC - 1, *fin.pop(BLOC - 1))

    nc.compile()
    return nc


def _get_nc():
    if "nc" not in _NC_CACHE:
        _NC_CACHE["nc"] = _build_nc()
    return _NC_CACHE["nc"]


def kernel(hidden_states, att_feats, p_att_feats, att_masks, W_h, W_alpha):
    from concourse.bass_utils import run_bass_kernel_spmd

    nc = _get_nc()
    hidden_states = np.ascontiguousarray(hidden_states, dtype=np.float32)
    att_feats = np.ascontiguousarray(att_feats, dtype=np.float32)
    p_att_feats = np.ascontiguousarray(p_att_feats, dtype=np.float32)
    att_masks = np.ascontiguousarray(att_masks, dtype=np.float32)
    W_h = np.ascontiguousarray(W_h, dtype=np.float32)
    W_alpha = np.ascontiguousarray(
        np.asarray(W_alpha, dtype=np.float32).reshape(1, A))

    whT = np.ascontiguousarray(W_h.T)                       # [H, A]
    nbc = BLOC + 1
    ebc = np.zeros((nbc, nbc * P), dtype=np.float32)
    for b in range(nbc):
        ebc[b, b * P:(b + 1) * P] = 1.0

    in_maps = []
    for i in range(NCORES):
        s = slice(i * BLOC, (i + 1) * BLOC)
        in_maps.append({
            "hidden_T": np.ascontiguousarray(hidden_states[s].T),
            "att_feats": att_feats[s],
            "p_att_feats": p_att_feats[s],
            "att_masks": att_masks[s],
            "W_hT": whT,
            "W_alpha_r": W_alpha,
            "E_bc": ebc,
        })

    global _LAST_IN_MAPS
    _LAST_IN_MAPS = in_maps
    res = run_bass_kernel_spmd(nc, in_maps, core_ids=list(range(NCORES)))
    return np.concatenate(
        [res.results[i]["att_out"] for i in range(NCORES)], axis=0
    ).astype(np.float32)


_LAST_IN_MAPS = None
